# revision 1
# baseline (speedup 1.0000x reference)
"""Trainium2 Bass kernel for CrossCondGPT2 block-sparse attention.

Contract: kernel(**inputs) takes the FULL unsharded inputs (as produced by
setup_inputs) and returns the FULL [B, L, C] float32 output.

Sharding: 8 cores = batch(4) x head-group(2). Core (b, g) computes heads
6g..6g+5 of batch b end-to-end (q/k/v projections, sparse attention, partial
output projection). The host sums the two head-group partials per batch and
adds the folded bias bp_eff = bp + Wp @ bv.

Mask structure (T=512, N=16, L=3T+5N=1616), at 128-tile granularity with
q-tiles r in 0..12 (tile 12 = 80 text rows) and k-tiles c likewise:
  r in 0..3   (motion block A): attends c in 0..r within A, causal diag.
  r in 4..7   (block B, rl=r-4): A cols c<=rl (causal diag), B cols c-4<=rl
              (strict diag), C cols c-8<=rl (strict diag), text (full).
  r in 8..11  (block C, rl=r-8): A cols c<=rl (causal), B cols c-4<=rl
              (causal), C cols c-8<=rl (strict), text (full).
  r = 12      (text): text only, full.

Attention is computed in S^T layout (keys on partitions, queries on free dim):
  ST = K_tile @ Q_tile^T via matmul(lhsT=Kt, rhs=Qt); softmax denominators come
from a ones-column appended to V (y_aug = exp(ST)^T-contracted with V_aug), so
no P-transposes and no max-subtraction are needed (logits ~ N(0,1)).
"""

import os
import sys

sys.path.insert(0, "/opt/trn_rl_repo")

import numpy as np
import ml_dtypes

import concourse.bass as bass
import concourse.tile as tile
from concourse import bacc, mybir
from concourse.bass_utils import run_bass_kernel_spmd

BF16 = mybir.dt.bfloat16
F32 = mybir.dt.float32
NPBF = ml_dtypes.bfloat16

L, C, T, HD = 1616, 768, 512, 64
NH, NHL = 12, 6          # total heads / heads per core
DL = NHL * HD            # 384 local head dims per core
LT = 13                  # 12 full 128-row tiles + one 80-row text tile
LTW = [128] * 12 + [80]
CK = C // 128            # 6 contraction chunks for C
MK = DL // 128           # 3 chunks for the local head dims
LCHUNKS = [(0, 512), (512, 512), (1024, 512), (1536, 80)]
NEG = -1.0e9
EXP_FUNC = mybir.ActivationFunctionType.Exp
ID_FUNC = mybir.ActivationFunctionType.Identity
COPY_FUNC = mybir.ActivationFunctionType.Copy


def _schedule():
    """Per q-tile r: list of ranges; each range is a list of (c, mask) with
    mask in {None, 'c' (causal diag), 's' (strict diag)}. Ranges are
    contiguous runs of k-tiles sharing one PSUM bank."""
    sched = []
    for r in range(4):
        sched.append([[(c, 'c' if c == r else None) for c in range(0, r + 1)]])
    for r in range(4, 8):
        rl = r - 4
        sched.append([
            [(c, 'c' if c == rl else None) for c in range(0, rl + 1)],
            [(c, 's' if c == r else None) for c in range(4, r + 1)],
            [(c, 's' if c == r + 4 else None) for c in range(8, r + 5)],
            [(12, None)],
        ])
    for r in range(8, 12):
        rl = r - 8
        sched.append([
            [(c, 'c' if c == rl else None) for c in range(0, rl + 1)],
            [(c, 'c' if c == r - 4 else None) for c in range(4, r - 3)],
            [(c, 's' if c == r else None) for c in range(8, r + 1)],
            [(12, None)],
        ])
    sched.append([[(12, None)]])
    return sched


SCHED = _schedule()

def build_nc():
    nc = bacc.Bacc("TRN2", target_bir_lowering=False, debug=False, num_devices=8)

    xT_d = nc.dram_tensor("xT", [C, L], BF16, kind="ExternalInput").ap()
    wq_d = nc.dram_tensor("wqT", [C, DL], BF16, kind="ExternalInput").ap()
    wk_d = nc.dram_tensor("wkT", [C, DL], BF16, kind="ExternalInput").ap()
    wv_d = nc.dram_tensor("wvT", [C, DL], BF16, kind="ExternalInput").ap()
    wp_d = nc.dram_tensor("wpT", [DL, C], BF16, kind="ExternalInput").ap()
    bq_d = nc.dram_tensor("bqs", [DL], F32, kind="ExternalInput").ap()
    bk_d = nc.dram_tensor("bks", [DL], F32, kind="ExternalInput").ap()
    # mask factors: diag-tile masks are applied on PE as an extra accumulated
    # matmul: blocked[k,q] = sum_j U[j,k] * V[j,q] with U = -1e9 * (shifted)
    # identity and V = lower-inclusive triangle of ones.
    uc_d = nc.dram_tensor("u_c", [128, 128], BF16, kind="ExternalInput").ap()
    us_d = nc.dram_tensor("u_s", [128, 128], BF16, kind="ExternalInput").ap()
    vt_d = nc.dram_tensor("v_tri", [128, 128], BF16, kind="ExternalInput").ap()
    out_d = nc.dram_tensor("out", [L, C], F32, kind="ExternalOutput").ap()

    with tile.TileContext(nc) as tc:
        with (
            tc.tile_pool(name="persist", bufs=1) as persist,
            tc.tile_pool(name="xin", bufs=3) as xin,
            tc.tile_pool(name="sbw", bufs=2) as sbw,
            tc.tile_pool(name="expp", bufs=6) as expp,
            tc.tile_pool(name="dramp", bufs=2, space="DRAM") as dramp,
            tc.tile_pool(name="psmain", bufs=2, space="PSUM") as ps_main,
            tc.tile_pool(name="psst", bufs=4, space="PSUM") as ps_st,
            tc.tile_pool(name="psyt", bufs=2, space="PSUM") as ps_yt,
        ):
            # ---- persistent SBUF: weights, mask factors, biases ----
            wq_sb = persist.tile([128, CK, DL], BF16)
            wk_sb = persist.tile([128, CK, DL], BF16)
            wv_sb = persist.tile([128, CK, DL], BF16)
            wq_r = wq_d.rearrange("(k p) n -> p k n", p=128)
            wk_r = wk_d.rearrange("(k p) n -> p k n", p=128)
            wv_r = wv_d.rearrange("(k p) n -> p k n", p=128)
            wp_sb = persist.tile([128, MK, C], BF16)
            bq_sb = persist.tile([128, MK], F32)
            bk_sb = persist.tile([128, MK], F32)
            uc_sb = persist.tile([128, 128], BF16)
            us_sb = persist.tile([128, 128], BF16)
            vt_sb = persist.tile([128, 128], BF16)

            xT = persist.tile([128, CK, L], BF16)
            qt = persist.tile([128, MK, L], BF16)
            kt = persist.tile([128, MK, L], BF16)
            vsb = persist.tile([128, LT, NHL, HD + 1], BF16)
            yt_all = persist.tile([128, MK, L], BF16)
            out_stage = persist.tile([128, LT, C], F32)
            ones_sb = persist.tile([1, 64], F32)
            nc.vector.memset(ones_sb[0:1, 0:64], 1.0)

            # ---- input DMAs, urgency-ordered across 3 queues: biases and
            # Q/K weight chunks gate the first projection, then xT lo-major
            # (so the first L-chunk's accumulation completes early), with V
            # weights interleaved; wp/mask factors are needed late.
            xT_r = xT_d.rearrange("(k p) n -> p k n", p=128)
            dma_engs = [nc.sync, nc.gpsimd, nc.scalar]
            nc.gpsimd.dma_start(bq_sb[:], bq_d.rearrange("(m p) -> p m", p=128))
            nc.gpsimd.dma_start(bk_sb[:], bk_d.rearrange("(m p) -> p m", p=128))
            # Few large DMAs on HWDGE queues (per-DMA setup cost dominates
            # with many small transfers): sync and scalar each carry ~1.9MB;
            # gpsimd (SWDGE) gets the late/small pieces.
            nc.sync.dma_start(wq_sb[:], wq_r[:, :, :])
            nc.scalar.dma_start(wk_sb[:], wk_r[:, :, :])
            nc.sync.dma_start(xT[:, 0:3, :], xT_r[:, 0:3, :])
            nc.scalar.dma_start(xT[:, 3:6, :], xT_r[:, 3:6, :])
            nc.gpsimd.dma_start(wv_sb[:], wv_r[:, :, :])
            nc.scalar.dma_start(uc_sb[:], uc_d[:])
            nc.scalar.dma_start(us_sb[:], us_d[:])
            nc.scalar.dma_start(vt_sb[:], vt_d[:])
            nc.sync.dma_start(wp_sb[:], wp_d.rearrange("(k p) n -> p k n", p=128))

            def proj_qk(m):
                """Qt/Kt chunk m (heads 2m, 2m+1): [128, L] each, bias+scale
                applied on DVE during PSUM evacuation."""
                for wsb, bsb, dst, scale in (
                    (wq_sb, bq_sb, qt, 0.125),
                    (wk_sb, bk_sb, kt, 1.0),
                ):
                    for lo, lwc in LCHUNKS:
                        pm = ps_main.tile([128, 512], F32, tag="mm")
                        for kk in range(CK):
                            nc.tensor.matmul(
                                pm[0:128, 0:lwc],
                                wsb[:, kk, m * 128:(m + 1) * 128],
                                xT[:, kk, lo:lo + lwc],
                                start=(kk == 0),
                                stop=(kk == CK - 1),
                            )
                        nc.vector.tensor_scalar(
                            dst[:, m, lo:lo + lwc],
                            pm[0:128, 0:lwc],
                            bsb[:, m:m + 1],
                            scale,
                            mybir.AluOpType.add,
                            mybir.AluOpType.mult,
                        )

            def proj_v():
                """V in natural layout [L, DL], per head with a ones column
                (65th) that accumulates softmax denominators in PV."""
                for lt in range(LT):
                    lw = LTW[lt]
                    pm = ps_main.tile([128, 512], F32, tag="mm")
                    for kk in range(CK):
                        nc.tensor.matmul(
                            pm[0:lw, 0:DL],
                            xT[:, kk, lt * 128:lt * 128 + lw],
                            wv_sb[:, kk, :],
                            start=(kk == 0),
                            stop=(kk == CK - 1),
                        )
                    nc.vector.tensor_copy(
                        vsb[0:lw, lt, :, 0:HD],
                        pm[0:lw, 0:DL].rearrange("p (h d) -> p h d", h=NHL),
                    )
                    nc.vector.memset(vsb[0:lw, lt, :, HD:HD + 1], 1.0)

            def attn_head(h):
                hc, ho = h // 2, (h % 2) * 64
                qh = qt[ho:ho + 64, hc, :]
                kh = kt[ho:ho + 64, hc, :]
                ystage = sbw.tile([65, L], F32, tag="ystage")
                for r in range(LT):
                    qw = LTW[r]
                    qsl = slice(r * 128, r * 128 + qw)
                    pairs = [p for rg in SCHED[r] for p in rg]
                    motion = [p for p in pairs if p[0] < 12]
                    text = [p for p in pairs if p[0] == 12]
                    banks = [motion[i:i + 4] for i in range(0, len(motion), 4)]
                    if text:
                        banks.append(text)
                    total_pairs = len(pairs)
                    ytp = ps_yt.tile([65, 128], F32, tag="yt")
                    pv_args = []
                    for bank in banks:
                        stp = ps_st.tile([128, 512], F32, tag="st")
                        exps = expp.tile([128, 512], BF16, tag="expst")
                        kw_b = 128
                        for i, (c, mk) in enumerate(bank):
                            kw = 128 if c < 12 else 80
                            kw_b = kw
                            sl = stp[0:kw, i * 128:i * 128 + qw]
                            nc.tensor.matmul(
                                sl, kh[:, c * 128:c * 128 + kw], qh[:, qsl],
                                start=True, stop=(mk is None),
                            )
                            if mk is not None:
                                nc.tensor.matmul(
                                    sl, uc_sb[:] if mk == 'c' else us_sb[:],
                                    vt_sb[0:128, 0:qw],
                                    start=False, stop=True,
                                )
                            pv_args.append((c, kw, exps, i))
                        ew = (len(bank) - 1) * 128 + qw
                        nc.scalar.activation(
                            exps[0:kw_b, 0:ew], stp[0:kw_b, 0:ew], EXP_FUNC)
                    for pv_i, (c, kw, exps, i) in enumerate(pv_args):
                        nc.tensor.matmul(
                            ytp[0:65, 0:qw],
                            vsb[0:kw, c, h, :],
                            exps[0:kw, i * 128:i * 128 + qw],
                            start=(pv_i == 0),
                            stop=(pv_i == total_pairs - 1),
                        )
                    nc.vector.tensor_copy(ystage[0:65, qsl], ytp[0:65, 0:qw])
                # normalize. DVE reciprocal runs at 8 cycles/element on 2
                # slices, so a [1, L] reciprocal costs ~13us on one lane;
                # instead bounce the sums row through DRAM reshaped to
                # [101, 16] (L = 101*16) so the reciprocal is
                # partition-parallel (~1us), write it back flat, then
                # partition-broadcast from DRAM and multiply on the
                # otherwise-idle GPSIMD writing yT in place. The last head's
                # multiply uses a PE outer-product + DVE multiply instead --
                # it sits on the critical path into the final
                # output-projection pass.
                # (DMA is a flat copy in iteration order, so a [1, L] row and
                # a [101, 16] tile correspond element-for-element.)
                srec = sbw.tile([101, 16], F32, tag="srec")
                nc.scalar.dma_start(srec[0:101, :], ystage[64:65, :])
                nc.vector.reciprocal(srec[0:101, :], srec[0:101, :])
                if h < NHL - 1:
                    rec_d = dramp.tile([1, L], F32, tag="recd")
                    nc.gpsimd.dma_start(rec_d[0:1, :], srec[0:101, :])
                    recb = sbw.tile([64, L], F32, tag="recb")
                    nc.gpsimd.dma_start(
                        recb[0:64, :], rec_d[0:1, :].to_broadcast((64, L)))
                    nc.gpsimd.tensor_mul(
                        yt_all[ho:ho + 64, hc, :], ystage[0:64, :],
                        recb[0:64, :])
                else:
                    rec_row = sbw.tile([1, L], F32, tag="recrow")
                    nc.sync.dma_start(rec_row[0:1, :], srec[0:101, :])
                    for lo, lwc in LCHUNKS:
                        bc = ps_yt.tile([64, 512], F32, tag="yt")
                        nc.tensor.matmul(
                            bc[0:64, 0:lwc], ones_sb[0:1, 0:64],
                            rec_row[0:1, lo:lo + lwc],
                            start=True, stop=True,
                        )
                        nc.vector.tensor_mul(
                            yt_all[ho:ho + 64, hc, lo:lo + lwc],
                            ystage[0:64, lo:lo + lwc], bc[0:64, 0:lwc])

            def outproj_a():
                """Output-projection partial over head-chunks 0,1 (heads
                0..3) into out_stage; runs while heads 4,5 compute."""
                for r in range(LT):
                    qw = LTW[r]
                    qsl = slice(r * 128, r * 128 + qw)
                    for no, nw in ((0, 512), (512, 256)):
                        pm = ps_main.tile([128, 512], F32, tag="mm")
                        for kk in (0, 1):
                            nc.tensor.matmul(
                                pm[0:qw, 0:nw],
                                yt_all[:, kk, qsl],
                                wp_sb[:, kk, no:no + nw],
                                start=(kk == 0),
                                stop=(kk == 1),
                            )
                        nc.vector.tensor_copy(
                            out_stage[0:qw, r, no:no + nw], pm[0:qw, 0:nw])

            def outproj_b():
                """Final head-chunk 2 contribution, added into out_stage on
                DVE, then written out."""
                for r in range(LT):
                    qw = LTW[r]
                    qsl = slice(r * 128, r * 128 + qw)
                    for no, nw in ((0, 512), (512, 256)):
                        pm = ps_main.tile([128, 512], F32, tag="mm")
                        nc.tensor.matmul(
                            pm[0:qw, 0:nw],
                            yt_all[:, 2, qsl],
                            wp_sb[:, 2, no:no + nw],
                            start=True, stop=True,
                        )
                        nc.vector.tensor_add(
                            out_stage[0:qw, r, no:no + nw],
                            out_stage[0:qw, r, no:no + nw],
                            pm[0:qw, 0:nw])
                    eng = nc.sync if r % 2 == 0 else nc.scalar
                    eng.dma_start(out_d[qsl, :], out_stage[0:qw, r, :])

            # ---- interleave projections with attention head-pairs so PE
            # fills the ACT-bound gaps of the attention phase ----
            proj_qk(0)
            proj_v()
            attn_head(0)
            proj_qk(1)
            attn_head(1)
            attn_head(2)
            proj_qk(2)
            attn_head(3)
            outproj_a()
            attn_head(4)
            attn_head(5)
            outproj_b()

    nc.compile()
    return nc


_NC_CACHE = None


def _get_nc():
    global _NC_CACHE
    if _NC_CACHE is None:
        _NC_CACHE = build_nc()
    return _NC_CACHE


def make_in_maps(inputs):
    x = np.asarray(inputs["x"], np.float32)
    Wq = np.asarray(inputs["Wq"], np.float32)
    Wk = np.asarray(inputs["Wk"], np.float32)
    Wv = np.asarray(inputs["Wv"], np.float32)
    Wp = np.asarray(inputs["Wp"], np.float32)
    bq = np.asarray(inputs["bq"], np.float32)
    bk = np.asarray(inputs["bk"], np.float32)

    # Diag-mask factors (ST layout [k, q]; blocked positions get -1e9 via a
    # PE-accumulated matmul U.T @ V): causal blocks q < k, strict blocks q <= k.
    u_c = (NEG * np.eye(128, k=1)).astype(NPBF)
    u_s = (NEG * np.eye(128)).astype(NPBF)
    v_tri = np.tril(np.ones((128, 128))).astype(NPBF)

    in_maps = []
    for b in range(4):
        xT_b = np.ascontiguousarray(x[b].T).astype(NPBF)
        for g in range(2):
            sl = slice(g * DL, (g + 1) * DL)
            in_maps.append({
                "xT": xT_b,
                "wqT": np.ascontiguousarray(Wq[sl, :].T).astype(NPBF),
                "wkT": np.ascontiguousarray(Wk[sl, :].T).astype(NPBF),
                "wvT": np.ascontiguousarray(Wv[sl, :].T).astype(NPBF),
                "wpT": np.ascontiguousarray(Wp[:, sl].T).astype(NPBF),
                "bqs": np.ascontiguousarray(bq[sl]).astype(np.float32),
                "bks": np.ascontiguousarray(bk[sl]).astype(np.float32),
                "u_c": u_c,
                "u_s": u_s,
                "v_tri": v_tri,
            })
    return in_maps


def combine_outputs(inputs, results):
    Wp = np.asarray(inputs["Wp"], np.float32)
    bv = np.asarray(inputs["bv"], np.float32)
    bp = np.asarray(inputs["bp"], np.float32)
    # V bias folds through attention (rows of P sum to 1) and the projection:
    # (y + bv) @ Wp.T + bp = y @ Wp.T + (bp + Wp @ bv)
    bp_eff = bp + Wp @ bv
    parts = [r["out"] for r in results]
    out = np.stack(
        [parts[2 * b] + parts[2 * b + 1] + bp_eff for b in range(4)]
    ).astype(np.float32)
    return out


def run(inputs, trace=False):
    nc = _get_nc()
    in_maps = make_in_maps(inputs)
    res = run_bass_kernel_spmd(
        nc, in_maps, core_ids=list(range(8)), trace=trace
    )
    return combine_outputs(inputs, res.results), res


def kernel(**inputs):
    out, _ = run(inputs, trace=False)
    return out

